# revision 2
# baseline (speedup 1.0000x reference)
"""Trainium2 Bass kernel for nn_NoisyActLin (fake-quantized linear layer).

y = x_dq @ w_dq.T + bias, where
  x_dq = per-tensor fake-quant of x   (scale s = 2^log_act_s, zero point zp)
  w_dq = per-out-channel fake-quant of w (scale 2^log_wght_s, min/max rounded
         to half-scale grid)

Strategy (8 NeuronCores, column-parallel / tensor-parallel):
  - shard weight/bias/log_wght_s along out_features (1024 rows per core)
  - replicate x (pre-transposed on host to [DIN, TOK] so the contraction dim
    lands on SBUF partitions with no on-chip transpose)
  - each core: quantize x (replicated work), quantize its weight shard,
    transpose w_dq on-chip via DMA-xbar, then matmul with f32 PSUM accum.
  - gather: concatenate per-core [TOK, 1024] outputs along the out dim.

Quantization decisions (round points) are bit-identical to the reference:
round-to-nearest-even is the +2^23 / +1.5*2^23 magic-number trick, with the
same intermediate f32 roundings as the reference (clip -> sub -> scale ->
round).

Matmul modes:
  - "fp32r": single pass, inputs rounded RNE to 12-bit significand by the
    hardware FP32R path at full bf16 throughput.  w_dq is exactly
    representable (8-bit significand); only |x_dq| >= 4 elements (~0.003%)
    are rounded, worst-case output error ~1.3e-4 on an output scale of 5.3.
  - "bf16x2": two passes with an exact hi/lo bf16 split of x_dq
    (x_dq = hi + lo exactly). Bit-exact output, 2x the PE work.
"""

import numpy as np

B, S, DIN, DOUT = 4, 2048, 2048, 8192
NCORES = 8
DSH = DOUT // NCORES          # 1024 out-features per core
TOK = B * S                   # 8192 tokens
KT = DIN // 128               # 16 K tiles
TSUP = 512                    # tokens per supertile
NT = TOK // TSUP              # 16 supertiles
MM = TSUP // 128              # 4 M (token) tiles per supertile
ND = DSH // 512               # 2 N (dout) tiles per core

ACT_GUARD = 2.0
WGT_GUARD = 2.0
MAGIC = 8388608.0             # 2^23: unsigned round-to-int magic
SMAGIC = 12582912.0           # 1.5*2^23: signed round-to-int magic

MODE = "fp32r"                # "fp32r" | "bf16x2"

_CACHE = {}


def _split_multi_waits(nc, mybir):
    """walrus in this container only accepts one sync-wait per instruction;
    Tile's kernel-tail drain can carry several (one per live DMA queue).
    Hoist extras onto dedicated NOPs."""
    for bb in nc.main_func.blocks:
        new_list = []
        for ins in bb.instructions:
            si = ins.sync_info
            if si is not None and si.on_wait and len(si.on_wait) > 1:
                waits = list(si.on_wait)
                for j, w in enumerate(waits[:-1]):
                    new_list.append(mybir.InstNoOp(
                        name=f"{ins.name}-wsplit-{j}",
                        sync_info=mybir.SyncInfo(on_wait=[w], on_update=[]),
                        bass_nofuse=True,
                        engine=ins.engine,
                    ))
                ins.sync_info = mybir.SyncInfo(
                    on_wait=[waits[-1]], on_update=list(si.on_update))
            new_list.append(ins)
        bb.instructions[:] = new_list


def _build(zp, inv_s, s, lo, hi, need_clip, mode=None, t_limit=NT):
    import concourse.bass as bass
    import concourse.tile as tile
    import concourse.mybir as mybir

    if mode is None:
        mode = MODE
    f32 = mybir.dt.float32
    bf16 = mybir.dt.bfloat16
    r32 = mybir.dt.float32r
    OP = mybir.AluOpType
    ACT = mybir.ActivationFunctionType

    # x_dq = qs * s + C  with  qs = round((x - zp) * inv_s) + 2^23
    C = float(np.float32(np.float32(zp) - np.float32(MAGIC) * np.float32(s)))

    nc = bass.Bass()
    xt = nc.dram_tensor("xt", [DIN, TOK], f32, kind="ExternalInput")
    w = nc.dram_tensor("w", [DSH, DIN], f32, kind="ExternalInput")
    wscale = nc.dram_tensor("wscale", [DSH], f32, kind="ExternalInput")
    winv = nc.dram_tensor("winv", [DSH], f32, kind="ExternalInput")
    bias = nc.dram_tensor("bias", [DSH], f32, kind="ExternalInput")
    y = nc.dram_tensor("y", [TOK, DSH], f32, kind="ExternalOutput")

    wT_dt = r32 if mode == "fp32r" else bf16

    with tile.TileContext(nc) as tc:
        with tc.tile_pool(name="persist", bufs=1) as persist:
            # persistent: transposed quantized weights + broadcast bias
            wT = persist.tile([128, KT, DSH], wT_dt, tag="wT")
            bias_bc = persist.tile([128, DSH], f32, tag="bias_bc")
            nc.sync.dma_start(
                bias_bc[:],
                bias[:].rearrange("(a b) -> a b", a=1).to_broadcast((128, DSH)))

            # ---- weight fake-quant + transpose, one [128, DIN] row-tile at a time
            with (
                tc.tile_pool(name="wtmp", bufs=2) as wtmp,
                tc.tile_pool(name="stats", bufs=2) as stats,
            ):
                for wi in range(DSH // 128):
                    wtile = wtmp.tile([128, DIN], f32, tag="wtile")
                    nc.sync.dma_start(wtile[:], w[wi * 128:(wi + 1) * 128, :])
                    sc = stats.tile([128, 1], f32, tag="sc")
                    iv = stats.tile([128, 1], f32, tag="iv")
                    nc.sync.dma_start(
                        sc[:], wscale[wi * 128:(wi + 1) * 128].rearrange("(a b) -> a b", b=1))
                    nc.sync.dma_start(
                        iv[:], winv[wi * 128:(wi + 1) * 128].rearrange("(a b) -> a b", b=1))

                    wmin = stats.tile([128, 1], f32, tag="wmin")
                    wmax = stats.tile([128, 1], f32, tag="wmax")
                    nc.vector.tensor_reduce(
                        wmin[:], wtile[:], axis=mybir.AxisListType.X, op=OP.min)
                    nc.vector.tensor_reduce(
                        wmax[:], wtile[:], axis=mybir.AxisListType.X, op=OP.max)

                    # qwmin = round(wmin / sc * 2) / 2 * sc  (all steps exact but round)
                    qwmin = stats.tile([128, 1], f32, tag="qwmin")
                    qwmax = stats.tile([128, 1], f32, tag="qwmax")
                    for src, dst in ((wmin, qwmin), (wmax, qwmax)):
                        nc.vector.tensor_scalar(dst[:], src[:], iv[:], 2.0, OP.mult, OP.mult)
                        nc.vector.tensor_scalar(dst[:], dst[:], SMAGIC, SMAGIC, OP.add, OP.subtract)
                        nc.vector.tensor_scalar(dst[:], dst[:], 0.5, sc[:], OP.mult, OP.mult)

                    # w_dq = round((clip(w, qwmin, qwmax) - qwmin) / sc) * sc + qwmin
                    wc = wtmp.tile([128, DIN], f32, tag="wc")
                    nc.vector.tensor_scalar(wc[:], wtile[:], qwmin[:], qwmax[:], OP.max, OP.min)
                    nc.vector.tensor_scalar(wc[:], wc[:], qwmin[:], iv[:], OP.subtract, OP.mult)
                    nc.vector.tensor_scalar(wc[:], wc[:], MAGIC, MAGIC, OP.add, OP.subtract)
                    wq = wtmp.tile([128, DIN], bf16, tag="wq")
                    nc.vector.tensor_scalar(wq[:], wc[:], sc[:], qwmin[:], OP.mult, OP.add)

                    # transpose [dout=128, din] -> [din, dout=128] in 128x128 blocks
                    if mode == "fp32r":
                        wTb = wtmp.tile([128, KT, 128], bf16, tag="wTb")
                        for k in range(KT):
                            nc.sync.dma_start_transpose(
                                wTb[:, k, :], wq[:, k * 128:(k + 1) * 128])
                        # widen to fp32r (exact: w_dq has an 8-bit significand)
                        nc.vector.tensor_copy(wT[:, :, wi * 128:(wi + 1) * 128], wTb[:])
                    else:
                        for k in range(KT):
                            nc.sync.dma_start_transpose(
                                wT[:, k, wi * 128:(wi + 1) * 128],
                                wq[:, k * 128:(k + 1) * 128])

            # ---- main loop: activation fake-quant + matmul
            with (
                tc.tile_pool(name="xin", bufs=4) as xin,
                tc.tile_pool(name="xq", bufs=3) as xq,
                tc.tile_pool(name="xhl", bufs=2) as xhl,
                tc.tile_pool(name="psum", bufs=8, space="PSUM") as psum_pool,
                tc.tile_pool(name="outs", bufs=4) as outs,
            ):
                for t in range(t_limit):
                    if mode == "fp32r":
                        xr_t = xhl.tile([128, KT, TSUP], r32, tag="hi")
                    else:
                        hi_t = xhl.tile([128, KT, TSUP], bf16, tag="hi")
                        lo_t = xhl.tile([128, KT, TSUP], bf16, tag="lo")
                    for k in range(KT):
                        xs = xin.tile([128, TSUP], f32, tag="xs")
                        nc.sync.dma_start(
                            xs[:], xt[k * 128:(k + 1) * 128, t * TSUP:(t + 1) * TSUP])
                        if need_clip:
                            nc.vector.tensor_scalar(xs[:], xs[:], lo, hi, OP.max, OP.min)
                        # t1 = x - zp   (separate rounding, matches reference)
                        t1 = xq.tile([128, TSUP], f32, tag="t1")
                        nc.scalar.activation(t1[:], xs[:], ACT.Copy, bias=-zp, scale=1.0)
                        # qs = RNE(t1 * inv_s) + 2^23   (single FMA rounding)
                        qs = xq.tile([128, TSUP], f32, tag="qs")
                        nc.scalar.activation(qs[:], t1[:], ACT.Copy, bias=MAGIC, scale=inv_s)
                        if mode == "fp32r":
                            # x_dq = qs * s + C (exact), rounded to fp32r on write
                            nc.vector.tensor_scalar(
                                xr_t[:, k, :], qs[:], s, C, OP.mult, OP.add)
                        else:
                            xdq = xq.tile([128, TSUP], f32, tag="xdq")
                            nc.vector.tensor_scalar(xdq[:], qs[:], s, C, OP.mult, OP.add)
                            # exact bf16 split: hi = bf16(x_dq), lo = x_dq - hi
                            nc.vector.tensor_copy(hi_t[:, k, :], xdq[:])
                            nc.vector.tensor_tensor(
                                lo_t[:, k, :], xdq[:], hi_t[:, k, :], OP.subtract)

                    for d in range(ND):
                        for m in range(MM):
                            ps = psum_pool.tile([128, 512], f32, tag="ps")
                            for k in range(KT):
                                if mode == "fp32r":
                                    nc.tensor.matmul(
                                        ps[:], xr_t[:, k, m * 128:(m + 1) * 128],
                                        wT[:, k, d * 512:(d + 1) * 512],
                                        start=(k == 0), stop=(k == KT - 1))
                                else:
                                    nc.tensor.matmul(
                                        ps[:], hi_t[:, k, m * 128:(m + 1) * 128],
                                        wT[:, k, d * 512:(d + 1) * 512],
                                        start=(k == 0), stop=False)
                                    nc.tensor.matmul(
                                        ps[:], lo_t[:, k, m * 128:(m + 1) * 128],
                                        wT[:, k, d * 512:(d + 1) * 512],
                                        start=False, stop=(k == KT - 1))
                            ob = outs.tile([128, 512], f32, tag="ob")
                            nc.vector.tensor_tensor(
                                ob[:], ps[:], bias_bc[:, d * 512:(d + 1) * 512], OP.add)
                            nc.sync.dma_start(
                                y[t * TSUP + m * 128: t * TSUP + (m + 1) * 128,
                                  d * 512:(d + 1) * 512], ob[:])

    _split_multi_waits(nc, mybir)
    return nc


def kernel(x, weight, bias, log_act_s, log_act_q, act_b, log_wght_s):
    from concourse.bass_utils import run_bass_kernel_spmd

    f32 = np.float32
    # --- host scalar math, replicating the reference's f32 op order exactly
    s = np.exp2(log_act_s.astype(f32))[0]
    q = np.exp2(log_act_q.astype(f32))[0]
    zp = (np.round(act_b.astype(f32)[0] / s * f32(ACT_GUARD)) / f32(ACT_GUARD)) * s
    clip_lo = zp
    clip_hi = (zp + q) - s
    inv_s = f32(1.0) / s
    assert inv_s == np.exp2(-log_act_s.astype(f32))[0]  # s is a power of two

    xf = x.reshape(TOK, DIN)
    xmin, xmax = float(xf.min()), float(xf.max())
    need_clip = (xmin < float(clip_lo)) or (xmax > float(clip_hi))

    xt = np.ascontiguousarray(xf.T)  # [DIN, TOK]

    wscale = np.exp2(log_wght_s.astype(f32))[:, 0]          # [DOUT]
    winv = f32(1.0) / wscale                                # exact: powers of two
    bias_f = bias.astype(f32)

    key = (float(zp), float(inv_s), float(s), float(clip_lo), float(clip_hi),
           need_clip, MODE)
    if key not in _CACHE:
        _CACHE[key] = _build(float(zp), float(inv_s), float(s),
                             float(clip_lo), float(clip_hi), need_clip)
    nc = _CACHE[key]

    in_maps = []
    for c in range(NCORES):
        sl = slice(c * DSH, (c + 1) * DSH)
        in_maps.append({
            "xt": xt,
            "w": np.ascontiguousarray(weight[sl]).astype(f32),
            "wscale": np.ascontiguousarray(wscale[sl]),
            "winv": np.ascontiguousarray(winv[sl]),
            "bias": np.ascontiguousarray(bias_f[sl]),
        })

    res = run_bass_kernel_spmd(nc, in_maps, core_ids=list(range(NCORES)))
    y = np.concatenate([res.results[c]["y"] for c in range(NCORES)], axis=1)
    return np.ascontiguousarray(y.reshape(B, S, DOUT))


# revision 3
# speedup vs baseline: 1.1467x; 1.1467x over previous
"""Trainium2 Bass kernel for nn_NoisyActLin (fake-quantized linear layer).

y = x_dq @ w_dq.T + bias, where
  x_dq = per-tensor fake-quant of x   (scale s = 2^log_act_s, zero point zp)
  w_dq = per-out-channel fake-quant of w (scale 2^log_wght_s, min/max rounded
         to half-scale grid)

Strategy (8 NeuronCores, column-parallel / tensor-parallel):
  - shard weight/bias/log_wght_s along out_features (1024 rows per core)
  - replicate x (pre-transposed on host to [DIN, TOK] so the contraction dim
    lands on SBUF partitions with no on-chip transpose)
  - each core: quantize x (replicated work), quantize its weight shard,
    transpose w_dq on-chip (PE transpose-mode via an identity matmul -- the
    DMA-xbar path serializes and costs ~140us here), then matmul with f32
    PSUM accumulation.
  - gather: concatenate per-core [TOK, 1024] outputs along the out dim.

Quantization decisions (round points) are bit-identical to the reference:
round-to-nearest-even is the +2^23 / +1.5*2^23 magic-number trick, with the
same intermediate f32 roundings as the reference (clip -> sub -> scale ->
round).

Matmul modes:
  - "fp32r": single pass at full bf16 throughput; the hardware FP32R path
    rounds inputs RNE to a 12-bit significand.  w_dq is exactly representable
    (8-bit significand); only |x_dq| >= 4 elements (~0.003% of x) get rounded.
    Measured output error vs the f32 reference: absmax 1.3e-4 on an output
    scale of 5.3 (rel-fro 5.1e-6).
  - "bf16x2": two passes with an exact hi/lo bf16 split of x_dq
    (x_dq = hi + lo exactly).  Bit-exact output, 2x the PE work.
"""

import numpy as np

B, S, DIN, DOUT = 4, 2048, 2048, 8192
NCORES = 8
DSH = DOUT // NCORES          # 1024 out-features per core
TOK = B * S                   # 8192 tokens
KT = DIN // 128               # 16 K tiles
TSUP = 512                    # tokens per supertile
NT = TOK // TSUP              # 16 supertiles
MM = TSUP // 128              # 4 M (token) tiles per supertile
ND = DSH // 512               # 2 N (dout) tiles per core

ACT_GUARD = 2.0
WGT_GUARD = 2.0
MAGIC = 8388608.0             # 2^23: unsigned round-to-int magic
SMAGIC = 12582912.0           # 1.5*2^23: signed round-to-int magic

MODE = "fp32r"                # "fp32r" | "bf16x2"

_CACHE = {}


def _split_multi_waits(nc, mybir):
    """walrus in this container only accepts one sync-wait per instruction;
    Tile's kernel-tail drain can carry several (one per live DMA queue).
    Hoist extras onto dedicated NOPs."""
    for bb in nc.main_func.blocks:
        new_list = []
        for ins in bb.instructions:
            si = ins.sync_info
            if si is not None and si.on_wait and len(si.on_wait) > 1:
                waits = list(si.on_wait)
                for j, w in enumerate(waits[:-1]):
                    new_list.append(mybir.InstNoOp(
                        name=f"{ins.name}-wsplit-{j}",
                        sync_info=mybir.SyncInfo(on_wait=[w], on_update=[]),
                        bass_nofuse=True,
                        engine=ins.engine,
                    ))
                ins.sync_info = mybir.SyncInfo(
                    on_wait=[waits[-1]], on_update=list(si.on_update))
            new_list.append(ins)
        bb.instructions[:] = new_list


def _build(zp, inv_s, s, lo, hi, need_clip, mode=None, t_limit=NT):
    import concourse.bass as bass
    import concourse.tile as tile
    import concourse.mybir as mybir
    from concourse.masks import make_identity

    if mode is None:
        mode = MODE
    f32 = mybir.dt.float32
    bf16 = mybir.dt.bfloat16
    r32 = mybir.dt.float32r
    OP = mybir.AluOpType
    ACT = mybir.ActivationFunctionType

    # x_dq = qs * s + C  with  qs = round((x - zp) * inv_s) + 2^23
    C = float(np.float32(np.float32(zp) - np.float32(MAGIC) * np.float32(s)))

    nc = bass.Bass()
    xt = nc.dram_tensor("xt", [DIN, TOK], f32, kind="ExternalInput")
    w = nc.dram_tensor("w", [DSH, DIN], f32, kind="ExternalInput")
    wscale = nc.dram_tensor("wscale", [DSH], f32, kind="ExternalInput")
    winv = nc.dram_tensor("winv", [DSH], f32, kind="ExternalInput")
    bias = nc.dram_tensor("bias", [DSH], f32, kind="ExternalInput")
    y = nc.dram_tensor("y", [TOK, DSH], f32, kind="ExternalOutput")

    wT_dt = r32 if mode == "fp32r" else bf16

    with tile.TileContext(nc) as tc:
        with tc.tile_pool(name="persist", bufs=1) as persist:
            # persistent: transposed quantized weights (split per output half)
            # + broadcast bias + identity for PE-mode transpose
            wTs = [persist.tile([128, KT, 512], wT_dt, tag=f"wT{d}", name=f"wT{d}")
                   for d in range(ND)]
            bias_bc = persist.tile([128, DSH], f32, tag="bias_bc")
            nc.sync.dma_start(
                bias_bc[:],
                bias[:].rearrange("(a b) -> a b", a=1).to_broadcast((128, DSH)))
            ident = persist.tile([128, 128], f32, tag="ident")
            if mode == "fp32r":
                make_identity(nc, ident[:])

            # ---- weight fake-quant + transpose, one [128, DIN] row-tile at a time
            with (
                tc.tile_pool(name="wtmp", bufs=2) as wtmp,
                tc.tile_pool(name="stats", bufs=2) as stats,
                tc.tile_pool(name="tpsum", bufs=4, space="PSUM") as tpsum,
            ):
                for wi in range(DSH // 128):
                    d_idx, wj = divmod(wi, DSH // 128 // ND)
                    wtile = wtmp.tile([128, DIN], f32, tag="wtile")
                    nc.sync.dma_start(wtile[:], w[wi * 128:(wi + 1) * 128, :])
                    sc = stats.tile([128, 1], f32, tag="sc")
                    iv = stats.tile([128, 1], f32, tag="iv")
                    nc.sync.dma_start(
                        sc[:], wscale[wi * 128:(wi + 1) * 128].rearrange("(a b) -> a b", b=1))
                    nc.sync.dma_start(
                        iv[:], winv[wi * 128:(wi + 1) * 128].rearrange("(a b) -> a b", b=1))

                    wmin = stats.tile([128, 1], f32, tag="wmin")
                    wmax = stats.tile([128, 1], f32, tag="wmax")
                    nc.vector.tensor_reduce(
                        wmin[:], wtile[:], axis=mybir.AxisListType.X, op=OP.min)
                    nc.vector.tensor_reduce(
                        wmax[:], wtile[:], axis=mybir.AxisListType.X, op=OP.max)

                    # qwmin = round(wmin / sc * 2) / 2 * sc  (all steps exact but round)
                    qwmin = stats.tile([128, 1], f32, tag="qwmin")
                    qwmax = stats.tile([128, 1], f32, tag="qwmax")
                    for src, dst in ((wmin, qwmin), (wmax, qwmax)):
                        nc.vector.tensor_scalar(dst[:], src[:], iv[:], 2.0, OP.mult, OP.mult)
                        nc.vector.tensor_scalar(dst[:], dst[:], SMAGIC, SMAGIC, OP.add, OP.subtract)
                        nc.vector.tensor_scalar(dst[:], dst[:], 0.5, sc[:], OP.mult, OP.mult)

                    # w_dq = round((clip(w, qwmin, qwmax) - qwmin) / sc) * sc + qwmin
                    wc = wtmp.tile([128, DIN], f32, tag="wc")
                    nc.vector.tensor_scalar(wc[:], wtile[:], qwmin[:], qwmax[:], OP.max, OP.min)
                    nc.vector.tensor_scalar(wc[:], wc[:], qwmin[:], iv[:], OP.subtract, OP.mult)
                    nc.vector.tensor_scalar(wc[:], wc[:], MAGIC, MAGIC, OP.add, OP.subtract)

                    if mode == "fp32r":
                        # dequant in f32; transpose on the (otherwise idle) PE;
                        # evict PSUM->SBUF as fp32r on the scalar engine.
                        nc.vector.tensor_scalar(wc[:], wc[:], sc[:], qwmin[:], OP.mult, OP.add)
                        for k in range(KT):
                            pt = tpsum.tile([128, 128], f32, tag="pt", name="pt")
                            nc.tensor.transpose(pt[:], wc[:, k * 128:(k + 1) * 128], ident[:])
                            nc.scalar.copy(wTs[d_idx][:, k, wj * 128:(wj + 1) * 128], pt[:])
                    else:
                        wq = wtmp.tile([128, DIN], bf16, tag="wq")
                        nc.vector.tensor_scalar(wq[:], wc[:], sc[:], qwmin[:], OP.mult, OP.add)
                        for k in range(KT):
                            nc.sync.dma_start_transpose(
                                wTs[d_idx][:, k, wj * 128:(wj + 1) * 128],
                                wq[:, k * 128:(k + 1) * 128])

            # ---- main loop: activation fake-quant + matmul
            with (
                tc.tile_pool(name="xin", bufs=4) as xin,
                tc.tile_pool(name="xq", bufs=3) as xq,
                tc.tile_pool(name="xhl", bufs=2) as xhl,
                tc.tile_pool(name="psum", bufs=8, space="PSUM") as psum_pool,
                tc.tile_pool(name="outs", bufs=4) as outs,
            ):
                for t in range(t_limit):
                    if mode == "fp32r":
                        xr_t = xhl.tile([128, KT, TSUP], r32, tag="hi", name="xr")
                    else:
                        hi_t = xhl.tile([128, KT, TSUP], bf16, tag="hi")
                        lo_t = xhl.tile([128, KT, TSUP], bf16, tag="lo")
                    for k in range(KT):
                        xs = xin.tile([128, TSUP], f32, tag="xs")
                        nc.sync.dma_start(
                            xs[:], xt[k * 128:(k + 1) * 128, t * TSUP:(t + 1) * TSUP])
                        if need_clip:
                            nc.vector.tensor_scalar(xs[:], xs[:], lo, hi, OP.max, OP.min)
                        # t1 = x - zp   (separate rounding, matches reference)
                        t1 = xq.tile([128, TSUP], f32, tag="t1")
                        nc.scalar.activation(t1[:], xs[:], ACT.Copy, bias=-zp, scale=1.0)
                        # qs = RNE(t1 * inv_s) + 2^23   (single FMA rounding)
                        qs = xq.tile([128, TSUP], f32, tag="qs")
                        nc.scalar.activation(qs[:], t1[:], ACT.Copy, bias=MAGIC, scale=inv_s)
                        if mode == "fp32r":
                            # x_dq = qs * s + C (exact), rounded to fp32r on write
                            nc.vector.tensor_scalar(
                                xr_t[:, k, :], qs[:], s, C, OP.mult, OP.add)
                        else:
                            xdq = xq.tile([128, TSUP], f32, tag="xdq")
                            nc.vector.tensor_scalar(xdq[:], qs[:], s, C, OP.mult, OP.add)
                            # exact bf16 split: hi = bf16(x_dq), lo = x_dq - hi
                            nc.vector.tensor_copy(hi_t[:, k, :], xdq[:])
                            nc.vector.tensor_tensor(
                                lo_t[:, k, :], xdq[:], hi_t[:, k, :], OP.subtract)

                    for d in range(ND):
                        for m in range(MM):
                            ps = psum_pool.tile([128, 512], f32, tag="ps")
                            for k in range(KT):
                                if mode == "fp32r":
                                    nc.tensor.matmul(
                                        ps[:], xr_t[:, k, m * 128:(m + 1) * 128],
                                        wTs[d][:, k, :],
                                        start=(k == 0), stop=(k == KT - 1))
                                else:
                                    nc.tensor.matmul(
                                        ps[:], hi_t[:, k, m * 128:(m + 1) * 128],
                                        wTs[d][:, k, :],
                                        start=(k == 0), stop=False)
                                    nc.tensor.matmul(
                                        ps[:], lo_t[:, k, m * 128:(m + 1) * 128],
                                        wTs[d][:, k, :],
                                        start=False, stop=(k == KT - 1))
                            ob = outs.tile([128, 512], f32, tag="ob")
                            nc.vector.tensor_tensor(
                                ob[:], ps[:], bias_bc[:, d * 512:(d + 1) * 512], OP.add)
                            nc.sync.dma_start(
                                y[t * TSUP + m * 128: t * TSUP + (m + 1) * 128,
                                  d * 512:(d + 1) * 512], ob[:])

    _split_multi_waits(nc, mybir)
    return nc


def kernel(x, weight, bias, log_act_s, log_act_q, act_b, log_wght_s):
    from concourse.bass_utils import run_bass_kernel_spmd

    f32 = np.float32
    # --- host scalar math, replicating the reference's f32 op order exactly
    s = np.exp2(log_act_s.astype(f32))[0]
    q = np.exp2(log_act_q.astype(f32))[0]
    zp = (np.round(act_b.astype(f32)[0] / s * f32(ACT_GUARD)) / f32(ACT_GUARD)) * s
    clip_lo = zp
    clip_hi = (zp + q) - s
    inv_s = f32(1.0) / s
    assert inv_s == np.exp2(-log_act_s.astype(f32))[0]  # s is a power of two

    xf = x.reshape(TOK, DIN)
    xmin, xmax = float(xf.min()), float(xf.max())
    need_clip = (xmin < float(clip_lo)) or (xmax > float(clip_hi))

    xt = np.ascontiguousarray(xf.T)  # [DIN, TOK]

    wscale = np.exp2(log_wght_s.astype(f32))[:, 0]          # [DOUT]
    winv = f32(1.0) / wscale                                # exact: powers of two
    bias_f = bias.astype(f32)

    key = (float(zp), float(inv_s), float(s), float(clip_lo), float(clip_hi),
           need_clip, MODE)
    if key not in _CACHE:
        _CACHE[key] = _build(float(zp), float(inv_s), float(s),
                             float(clip_lo), float(clip_hi), need_clip)
    nc = _CACHE[key]

    in_maps = []
    for c in range(NCORES):
        sl = slice(c * DSH, (c + 1) * DSH)
        in_maps.append({
            "xt": xt,
            "w": np.ascontiguousarray(weight[sl]).astype(f32),
            "wscale": np.ascontiguousarray(wscale[sl]),
            "winv": np.ascontiguousarray(winv[sl]),
            "bias": np.ascontiguousarray(bias_f[sl]),
        })

    res = run_bass_kernel_spmd(nc, in_maps, core_ids=list(range(NCORES)))
    y = np.concatenate([res.results[c]["y"] for c in range(NCORES)], axis=1)
    return np.ascontiguousarray(y.reshape(B, S, DOUT))


# revision 6
# speedup vs baseline: 1.1931x; 1.0405x over previous
"""Trainium2 Bass kernel for nn_NoisyActLin (fake-quantized linear layer).

y = x_dq @ w_dq.T + bias, where
  x_dq = per-tensor fake-quant of x   (scale s = 2^log_act_s, zero point zp)
  w_dq = per-out-channel fake-quant of w (scale 2^log_wght_s, min/max rounded
         to half-scale grid)

Strategy (8 NeuronCores, column-parallel / tensor-parallel):
  - shard weight/bias/log_wght_s along out_features (1024 rows per core)
  - replicate x (pre-transposed on host to [DIN, TOK] so the contraction dim
    lands on SBUF partitions with no on-chip transpose)
  - each core: quantize x (replicated work), quantize its weight shard,
    transpose w_dq on-chip (PE transpose-mode via an identity matmul -- the
    DMA-xbar path serializes and costs ~140us here), then matmul with f32
    PSUM accumulation.
  - gather: concatenate per-core [TOK, 1024] outputs along the out dim.

Quantization decisions (round points) are bit-identical to the reference:
round-to-nearest-even is the +2^23 / +1.5*2^23 magic-number trick, with the
same intermediate f32 roundings as the reference (clip -> sub -> scale ->
round).

Matmul modes:
  - "fp32r": single pass at full bf16 throughput; the hardware FP32R path
    rounds inputs RNE to a 12-bit significand.  w_dq is exactly representable
    (8-bit significand); only |x_dq| >= 4 elements (~0.003% of x) get rounded.
    Measured output error vs the f32 reference: absmax 1.3e-4 on an output
    scale of 5.3 (rel-fro 5.1e-6).
  - "bf16x2": two passes with an exact hi/lo bf16 split of x_dq
    (x_dq = hi + lo exactly).  Bit-exact output, 2x the PE work.
"""

import numpy as np

B, S, DIN, DOUT = 4, 2048, 2048, 8192
NCORES = 8
DSH = DOUT // NCORES          # 1024 out-features per core
TOK = B * S                   # 8192 tokens
KT = DIN // 128               # 16 K tiles
TSUP = 512                    # tokens per supertile
NT = TOK // TSUP              # 16 supertiles
MM = TSUP // 128              # 4 M (token) tiles per supertile
ND = DSH // 512               # 2 N (dout) tiles per core

ACT_GUARD = 2.0
WGT_GUARD = 2.0
MAGIC = 8388608.0             # 2^23: unsigned round-to-int magic
SMAGIC = 12582912.0           # 1.5*2^23: signed round-to-int magic

MODE = "fp32r"                # "fp32r" | "bf16x2"

_CACHE = {}


def _split_multi_waits(nc, mybir):
    """walrus in this container only accepts one sync-wait per instruction;
    Tile's kernel-tail drain can carry several (one per live DMA queue).
    Hoist extras onto dedicated NOPs."""
    for bb in nc.main_func.blocks:
        new_list = []
        for ins in bb.instructions:
            si = ins.sync_info
            if si is not None and si.on_wait and len(si.on_wait) > 1:
                waits = list(si.on_wait)
                for j, w in enumerate(waits[:-1]):
                    new_list.append(mybir.InstNoOp(
                        name=f"{ins.name}-wsplit-{j}",
                        sync_info=mybir.SyncInfo(on_wait=[w], on_update=[]),
                        bass_nofuse=True,
                        engine=ins.engine,
                    ))
                ins.sync_info = mybir.SyncInfo(
                    on_wait=[waits[-1]], on_update=list(si.on_update))
            new_list.append(ins)
        bb.instructions[:] = new_list


def _build(zp, inv_s, s, lo, hi, need_clip, mode=None, t_limit=NT):
    import concourse.bass as bass
    import concourse.tile as tile
    import concourse.mybir as mybir
    from concourse.masks import make_identity

    if mode is None:
        mode = MODE
    f32 = mybir.dt.float32
    bf16 = mybir.dt.bfloat16
    r32 = mybir.dt.float32r
    OP = mybir.AluOpType
    ACT = mybir.ActivationFunctionType

    # x_dq = qs * s + C  with  qs = round((x - zp) * inv_s) + 2^23
    C = float(np.float32(np.float32(zp) - np.float32(MAGIC) * np.float32(s)))

    nc = bass.Bass()
    xt = nc.dram_tensor("xt", [DIN, TOK], f32, kind="ExternalInput")
    w = nc.dram_tensor("w", [DSH, DIN], f32, kind="ExternalInput")
    wscale = nc.dram_tensor("wscale", [DSH], f32, kind="ExternalInput")
    winv = nc.dram_tensor("winv", [DSH], f32, kind="ExternalInput")
    bias = nc.dram_tensor("bias", [DSH], f32, kind="ExternalInput")
    y = nc.dram_tensor("y", [TOK, DSH], f32, kind="ExternalOutput")

    wT_dt = r32 if mode == "fp32r" else bf16

    with tile.TileContext(nc) as tc:
        with (
            tc.tile_pool(name="persist", bufs=1) as persist,
            tc.tile_pool(name="wtmp", bufs=2) as wtmp,
            tc.tile_pool(name="stats", bufs=2) as stats,
            tc.tile_pool(name="xin", bufs=4) as xin,
            tc.tile_pool(name="xq", bufs=3) as xq,
            tc.tile_pool(name="xhl", bufs=2) as xhl,
            tc.tile_pool(name="psum", bufs=4, space="PSUM") as psum_pool,
            tc.tile_pool(name="outs", bufs=4) as outs,
        ):
            # persistent: transposed quantized weights (split per output half)
            # + broadcast bias + identity for PE-mode transpose
            wTs = [persist.tile([128, KT, 512], wT_dt, tag=f"wT{d}", name=f"wT{d}")
                   for d in range(ND)]
            bias_bc = persist.tile([128, DSH], f32, tag="bias_bc")
            nc.sync.dma_start(
                bias_bc[:],
                bias[:].rearrange("(a b) -> a b", a=1).to_broadcast((128, DSH)))
            ident = persist.tile([128, 128], f32, tag="ident")
            if mode == "fp32r":
                make_identity(nc, ident[:])

            # ---- weight fake-quant + transpose, one [128, DIN] row-tile at a time
            if True:
                for wi in range(DSH // 128):
                    d_idx, wj = divmod(wi, DSH // 128 // ND)
                    wtile = wtmp.tile([128, DIN], f32, tag="wtile")
                    nc.sync.dma_start(wtile[:], w[wi * 128:(wi + 1) * 128, :])
                    sc = stats.tile([128, 1], f32, tag="sc")
                    iv = stats.tile([128, 1], f32, tag="iv")
                    nc.sync.dma_start(
                        sc[:], wscale[wi * 128:(wi + 1) * 128].rearrange("(a b) -> a b", b=1))
                    nc.sync.dma_start(
                        iv[:], winv[wi * 128:(wi + 1) * 128].rearrange("(a b) -> a b", b=1))

                    wmin = stats.tile([128, 1], f32, tag="wmin")
                    wmax = stats.tile([128, 1], f32, tag="wmax")
                    nc.vector.tensor_reduce(
                        wmin[:], wtile[:], axis=mybir.AxisListType.X, op=OP.min)
                    nc.vector.tensor_reduce(
                        wmax[:], wtile[:], axis=mybir.AxisListType.X, op=OP.max)

                    # qwmin = round(wmin / sc * 2) / 2 * sc  (all steps exact but round)
                    qwmin = stats.tile([128, 1], f32, tag="qwmin")
                    qwmax = stats.tile([128, 1], f32, tag="qwmax")
                    for src, dst in ((wmin, qwmin), (wmax, qwmax)):
                        nc.vector.tensor_scalar(dst[:], src[:], iv[:], 2.0, OP.mult, OP.mult)
                        nc.vector.tensor_scalar(dst[:], dst[:], SMAGIC, SMAGIC, OP.add, OP.subtract)
                        nc.vector.tensor_scalar(dst[:], dst[:], 0.5, sc[:], OP.mult, OP.mult)

                    # w_dq = round((clip(w, qwmin, qwmax) - qwmin) / sc) * sc + qwmin
                    wc = wtmp.tile([128, DIN], f32, tag="wc")
                    nc.vector.tensor_scalar(wc[:], wtile[:], qwmin[:], qwmax[:], OP.max, OP.min)
                    nc.vector.tensor_scalar(wc[:], wc[:], qwmin[:], iv[:], OP.subtract, OP.mult)
                    nc.vector.tensor_scalar(wc[:], wc[:], MAGIC, MAGIC, OP.add, OP.subtract)

                    if mode == "fp32r":
                        # dequant in f32; transpose on the (otherwise idle) PE;
                        # evict PSUM->SBUF as fp32r on the scalar engine.
                        nc.vector.tensor_scalar(wc[:], wc[:], sc[:], qwmin[:], OP.mult, OP.add)
                        for k in range(KT):
                            pt = psum_pool.tile([128, 128], f32, tag="pt",
                                                name="pt", bufs=4)
                            nc.tensor.transpose(pt[:], wc[:, k * 128:(k + 1) * 128], ident[:])
                            nc.scalar.copy(wTs[d_idx][:, k, wj * 128:(wj + 1) * 128], pt[:])
                    else:
                        wq = wtmp.tile([128, DIN], bf16, tag="wq")
                        nc.vector.tensor_scalar(wq[:], wc[:], sc[:], qwmin[:], OP.mult, OP.add)
                        for k in range(KT):
                            nc.sync.dma_start_transpose(
                                wTs[d_idx][:, k, wj * 128:(wj + 1) * 128],
                                wq[:, k * 128:(k + 1) * 128])

            # ---- main loop: activation fake-quant + matmul
            if True:
                for t in range(t_limit):
                    if mode == "fp32r":
                        xr_t = xhl.tile([128, KT, TSUP], r32, tag="hi", name="xr")
                    else:
                        hi_t = xhl.tile([128, KT, TSUP], bf16, tag="hi")
                        lo_t = xhl.tile([128, KT, TSUP], bf16, tag="lo")
                    for k in range(KT):
                        xs = xin.tile([128, TSUP], f32, tag="xs")
                        nc.sync.dma_start(
                            xs[:], xt[k * 128:(k + 1) * 128, t * TSUP:(t + 1) * TSUP])
                        if need_clip:
                            nc.vector.tensor_scalar(xs[:], xs[:], lo, hi, OP.max, OP.min)
                        # t1 = x - zp   (separate rounding, matches reference)
                        t1 = xq.tile([128, TSUP], f32, tag="t1")
                        nc.scalar.activation(t1[:], xs[:], ACT.Copy, bias=-zp, scale=1.0)
                        # qs = RNE(t1 * inv_s) + 2^23   (single FMA rounding)
                        qs = xq.tile([128, TSUP], f32, tag="qs")
                        nc.scalar.activation(qs[:], t1[:], ACT.Copy, bias=MAGIC, scale=inv_s)
                        if mode == "fp32r":
                            # x_dq = qs * s + C (exact), rounded to fp32r on write
                            nc.vector.tensor_scalar(
                                xr_t[:, k, :], qs[:], s, C, OP.mult, OP.add)
                        else:
                            xdq = xq.tile([128, TSUP], f32, tag="xdq")
                            nc.vector.tensor_scalar(xdq[:], qs[:], s, C, OP.mult, OP.add)
                            # exact bf16 split: hi = bf16(x_dq), lo = x_dq - hi
                            nc.vector.tensor_copy(hi_t[:, k, :], xdq[:])
                            nc.vector.tensor_tensor(
                                lo_t[:, k, :], xdq[:], hi_t[:, k, :], OP.subtract)

                    for d in range(ND):
                        for m in range(MM):
                            ps = psum_pool.tile([128, 512], f32, tag="ps")
                            for k in range(KT):
                                if mode == "fp32r":
                                    nc.tensor.matmul(
                                        ps[:], xr_t[:, k, m * 128:(m + 1) * 128],
                                        wTs[d][:, k, :],
                                        start=(k == 0), stop=(k == KT - 1))
                                else:
                                    nc.tensor.matmul(
                                        ps[:], hi_t[:, k, m * 128:(m + 1) * 128],
                                        wTs[d][:, k, :],
                                        start=(k == 0), stop=False)
                                    nc.tensor.matmul(
                                        ps[:], lo_t[:, k, m * 128:(m + 1) * 128],
                                        wTs[d][:, k, :],
                                        start=False, stop=(k == KT - 1))
                            ob = outs.tile([128, 512], f32, tag="ob")
                            nc.vector.tensor_tensor(
                                ob[:], ps[:], bias_bc[:, d * 512:(d + 1) * 512], OP.add)
                            nc.sync.dma_start(
                                y[t * TSUP + m * 128: t * TSUP + (m + 1) * 128,
                                  d * 512:(d + 1) * 512], ob[:])

    _split_multi_waits(nc, mybir)
    return nc


def kernel(x, weight, bias, log_act_s, log_act_q, act_b, log_wght_s):
    from concourse.bass_utils import run_bass_kernel_spmd

    f32 = np.float32
    # --- host scalar math, replicating the reference's f32 op order exactly
    s = np.exp2(log_act_s.astype(f32))[0]
    q = np.exp2(log_act_q.astype(f32))[0]
    zp = (np.round(act_b.astype(f32)[0] / s * f32(ACT_GUARD)) / f32(ACT_GUARD)) * s
    clip_lo = zp
    clip_hi = (zp + q) - s
    inv_s = f32(1.0) / s
    assert inv_s == np.exp2(-log_act_s.astype(f32))[0]  # s is a power of two

    xf = x.reshape(TOK, DIN)
    xmin, xmax = float(xf.min()), float(xf.max())
    need_clip = (xmin < float(clip_lo)) or (xmax > float(clip_hi))

    xt = np.ascontiguousarray(xf.T)  # [DIN, TOK]

    wscale = np.exp2(log_wght_s.astype(f32))[:, 0]          # [DOUT]
    winv = f32(1.0) / wscale                                # exact: powers of two
    bias_f = bias.astype(f32)

    key = (float(zp), float(inv_s), float(s), float(clip_lo), float(clip_hi),
           need_clip, MODE)
    if key not in _CACHE:
        _CACHE[key] = _build(float(zp), float(inv_s), float(s),
                             float(clip_lo), float(clip_hi), need_clip)
    nc = _CACHE[key]

    in_maps = []
    for c in range(NCORES):
        sl = slice(c * DSH, (c + 1) * DSH)
        in_maps.append({
            "xt": xt,
            "w": np.ascontiguousarray(weight[sl]).astype(f32),
            "wscale": np.ascontiguousarray(wscale[sl]),
            "winv": np.ascontiguousarray(winv[sl]),
            "bias": np.ascontiguousarray(bias_f[sl]),
        })

    res = run_bass_kernel_spmd(nc, in_maps, core_ids=list(range(NCORES)))
    y = np.concatenate([res.results[c]["y"] for c in range(NCORES)], axis=1)
    return np.ascontiguousarray(y.reshape(B, S, DOUT))
